# revision 27
# baseline (speedup 1.0000x reference)
"""Boundary rendering module for Trainium2 (8 NeuronCores), single-launch.

Computes, for x of shape (2, 4, 64, 256, 256) f32:
    mn/mx  = per-channel global min/max
    binary = ((x - mn) / (mx - mn)) > 0.5     [== x > (mn + mx)/2]
    dilated = 3x3x3 binary dilation of binary (SAME padding)
    out    = dilated - binary

Sharding: H (=256) split into 8 chunks of 32 rows, one per NeuronCore.
Each core receives its 32 rows plus one halo row on each side (global
edges padded with -1e30 so the halo mask is 0).  On-core layout puts
(B, D) = 128 on the SBUF partition axis; (C, H, W) live on the free axis.

Single NEFF:
  phase 1: SWDGE loads in 8-row packets (17KB packets run 5x slower than
  8KB on the SDMA read path), DVE min/max reduces interleaved per chunk,
  PE-transpose to 8 partitions, [mx(4) | -mn(4)] replicated 8x and
  exchanged with a one-hop mesh AllToAll (a ring AllReduce costs ~57us
  for 32B; AllToAll is direct), local max over cores, rank-1 PE
  broadcast.
  phase 2 per (channel, 8-row quarter): threshold on the Scalar engine
  (saturated sigmoid at scale 1e8 -> exact {0,1}), H-dilate on DVE,
  W-dilate on DVE (even quarters) or folded into the PE dw-shifted
  band matmuls (odd quarters), D-window count + -16*binary in PSUM,
  saturated sigmoid -> out, SWDGE stores.
"""

import os
import sys

import numpy as np

for _p in ("/opt/trn_rl_repo", "/root/.axon_site/_ro/trn_rl_repo"):
    if os.path.isdir(_p) and _p not in sys.path:
        sys.path.insert(0, _p)

import ml_dtypes

B, C, D, H, W = 2, 4, 64, 256, 256
NCORES = 8
HS = H // NCORES  # 32 own rows per core
HA = HS + 2  # rows incl halo
HPAD = np.float32(-1e30)  # halo pad at global H edges -> mask 0

# load chunks: strictly <=8KB per-partition packets (9KB packets run 3x
# slower on the SDMA read path); reduce chunks cover own rows 1..32 only
LROWS = [(0, 8), (8, 16), (16, 24), (24, 32), (32, 34)]
RROWS = [(1, 8), (8, 16), (16, 24), (24, 32), (32, 33)]

_CACHE = {}


def _consts():
    bd = np.arange(128)
    b = bd // D
    d = bd % D
    A = (b[:, None] == b[None, :]) & (np.abs(d[:, None] - d[None, :]) <= 1)
    A = A.astype(ml_dtypes.bfloat16)
    negI = (-16.0 * np.eye(128)).astype(ml_dtypes.bfloat16)
    I128 = np.eye(128, dtype=np.float32)
    return A, negI, I128


W_PE_DEFAULT = frozenset({1, 2, 4, 6, 9, 12, 14})  # W-dilate on PE (7/16)
T_DVE_DEFAULT = frozenset({0, 3, 8, 11})  # threshold on DVE (4/16)


def _build(variant: str = "full", w_pe=W_PE_DEFAULT, t_dve=T_DVE_DEFAULT):
    import concourse.bass as bass
    import concourse.bacc as bacc
    import concourse.mybir as mybir
    import concourse.tile as tile
    from contextlib import ExitStack

    f32 = mybir.dt.float32
    bf16 = mybir.dt.bfloat16
    Alu = mybir.AluOpType
    Act = mybir.ActivationFunctionType

    nc = bacc.Bacc(
        "TRN2",
        target_bir_lowering=False,
        debug=False,
        num_devices=NCORES,
    )

    xs = nc.dram_tensor("xs", [B, C, D, HA, W], f32, kind="ExternalInput")
    out = nc.dram_tensor("out", [B, C, D, HS, W], f32, kind="ExternalOutput")
    A_np, negI_np, I_np = _consts()
    bandA_d = nc.inline_tensor(A_np, name="bandA")
    negI_d = nc.inline_tensor(negI_np, name="negI")
    ident_d = nc.inline_tensor(I_np, name="ident")

    xsa = xs.ap()
    outa = out.ap()

    with ExitStack() as ctx:
        tc = ctx.enter_context(tile.TileContext(nc))
        pers = ctx.enter_context(tc.tile_pool(name="pers", bufs=1))
        binp = ctx.enter_context(tc.tile_pool(name="binp", bufs=3))
        mwp = ctx.enter_context(tc.tile_pool(name="mwp", bufs=2))
        sgp = ctx.enter_context(tc.tile_pool(name="sgp", bufs=2))
        # deeper mask double-buffering: 3 mh tiles cycle so DVE can run
        # ahead of PE consumers without write-after-read stalls
        psump = ctx.enter_context(tc.tile_pool(name="psum", bufs=2, space="PSUM"))
        dram = ctx.enter_context(tc.tile_pool(name="dram", bufs=1, space="DRAM"))

        x_all = pers.tile([128, C, HA, W], f32)  # 136 KiB / partition
        # H-dilated mask, double-buffered manually: rows of 258 with zero
        # pad cols 0 and 257 so the W-shift views read zeros at the edges
        mh0 = pers.tile([128, 8, 258], bf16)
        mh1 = pers.tile([128, 8, 258], bf16)
        mh2 = pers.tile([128, 8, 258], bf16)
        mh = [mh0, mh1, mh2]
        pmax = pers.tile([128, 20], f32)
        pmin = pers.tile([128, 20], f32)
        red8 = pers.tile([128, 8], f32)  # [mx(4) | -mn(4)] local
        s8 = pers.tile([128, 1], f32)  # per-partition reduced (parts 0..7)
        s64 = pers.tile([128, 8], f32)  # s8 replicated 8x along free axis
        z8 = pers.tile([128, 8], f32)  # zeros
        s1v = pers.tile([128, 72], f32)  # gathered (0:64) + reduced (64:72)
        gv8 = pers.tile([128, 8], f32)  # broadcast [mx | -mn] on all parts
        mnv = pers.tile([128, 4], f32)  # mn per channel
        h4 = pers.tile([128, 4], f32)  # 0.5*(mx-mn) per channel
        bias4 = pers.tile([128, 4], f32)  # -1e8 * (mn + h) per channel
        At = pers.tile([128, 128], bf16)
        Nt = pers.tile([128, 128], bf16)
        It = pers.tile([128, 128], f32)
        ones1 = pers.tile([128, 128], f32)  # row 0 used as all-ones lhsT
        selb = pers.tile([128, 1], f32)

        ccin = dram.tile([8, 8], f32)
        ccout = dram.tile([8, 8], f32)

        nc.vector.memset(selb[:, :], -100.0)
        nc.vector.memset(ones1[:, :], 1.0)
        nc.vector.memset(z8[:, :], 0.0)
        nc.vector.memset(mh0[:, :, :], 0.0)
        nc.vector.memset(mh1[:, :, :], 0.0)
        nc.vector.memset(mh2[:, :, :], 0.0)
        nc.sync.dma_start(out=At[:, :], in_=bandA_d.ap())
        nc.sync.dma_start(out=Nt[:, :], in_=negI_d.ap())
        nc.sync.dma_start(out=It[:, :], in_=ident_d.ap())

        # ---- phase 1: load + global min/max, interleaved per chunk ----
        skip_p1 = variant == "p2"
        NG = len(LROWS)
        for c in range(C):
            for g in range(NG):
                l0, l1 = LROWS[g]
                nc.gpsimd.dma_start(
                    out=x_all[:, c, l0:l1, :],
                    in_=xsa[:, c, :, l0:l1, :],
                )
                if skip_p1:
                    continue
                r0, r1 = RROWS[g]
                k = NG * c + g
                chunk = x_all[:, c, r0:r1, :]
                nc.vector.tensor_reduce(
                    out=pmax[:, k : k + 1],
                    in_=chunk,
                    axis=mybir.AxisListType.XY,
                    op=Alu.max,
                )
                nc.vector.tensor_reduce(
                    out=pmin[:, k : k + 1],
                    in_=chunk,
                    axis=mybir.AxisListType.XY,
                    op=Alu.min,
                )
        if skip_p1:
            nc.vector.memset(pmax[:, :], 5.0)
            nc.vector.memset(pmin[:, :], -5.0)
        for c in range(C):
            nc.vector.tensor_reduce(
                out=red8[:, c : c + 1],
                in_=pmax[:, NG * c : NG * c + NG],
                axis=mybir.AxisListType.X,
                op=Alu.max,
            )
            nc.vector.tensor_reduce(
                out=red8[:, 4 + c : 5 + c],
                in_=pmin[:, NG * c : NG * c + NG],
                axis=mybir.AxisListType.X,
                op=Alu.min,
            )
        # negate mins so a single max combines both after the exchange
        nc.vector.tensor_scalar_mul(red8[:, 4:8], red8[:, 4:8], -1.0)
        # cross-partition max: transpose [128p, 8] -> psum [8p, 128] via PE
        pst = psump.tile([128, 2048], f32, tag="ps")
        nc.tensor.matmul(pst[0:8, 0:128], red8[:, :], It[:, :], start=True, stop=True)
        nc.vector.tensor_reduce(
            out=s8[0:8, 0:1],
            in_=pst[0:8, 0:128],
            axis=mybir.AxisListType.X,
            op=Alu.max,
        )
        # replicate the 8 values 8x along the free axis (one copy per peer)
        nc.vector.tensor_scalar(
            out=s64[0:8, 0:8],
            in0=z8[0:8, 0:8],
            scalar1=s8[0:8, 0:1],
            scalar2=None,
            op0=Alu.add,
        )
        # one-hop mesh AllToAll: ccin[j, v] = myvals[v] -> ccout[k, v] =
        # core k's vals[v]; local max over k replaces a 14-hop ring
        nc.sync.dma_start(
            out=ccin[:, :].rearrange("j v -> v j"), in_=s64[0:8, 0:8]
        )
        if variant in ("nocc", "p2"):
            nc.gpsimd.dma_start(out=ccout[:, :], in_=ccin[:, :])
        else:
            nc.gpsimd.collective_compute(
                "AllToAll",
                Alu.bypass,
                replica_groups=[list(range(NCORES))],
                ins=[ccin.opt()],
                outs=[ccout.opt()],
            )
        nc.sync.dma_start(
            out=s1v[0:1, 0:64], in_=ccout[:, :].rearrange("k v -> (k v)")[None, :]
        )
        nc.vector.tensor_reduce(
            out=s1v[0:1, 64:72],
            in_=s1v[0:1, 0:64].rearrange("p (k v) -> p v k", k=NCORES),
            axis=mybir.AxisListType.X,
            op=Alu.max,
        )
        # broadcast to 128 partitions with a rank-1 matmul
        psb = psump.tile([128, 2048], f32, tag="ps")
        nc.tensor.matmul(
            psb[:, 0:8], ones1[0:1, :], s1v[0:1, 64:72], start=True, stop=True
        )
        nc.vector.tensor_copy(gv8[:, :], psb[:, 0:8])
        nc.vector.tensor_scalar_mul(mnv[:, :], gv8[:, 4:8], -1.0)
        nc.vector.tensor_add(h4[:, :], gv8[:, 0:4], gv8[:, 4:8])
        nc.vector.tensor_scalar_mul(h4[:, :], h4[:, :], 0.5)
        # threshold bias for the scalar engine: sigmoid(1e8*(x - (mn+h)))
        nc.vector.tensor_add(bias4[:, :], mnv[:, :], h4[:, :])
        nc.vector.tensor_scalar_mul(bias4[:, :], bias4[:, :], -1.0e8)
        if variant == "dbg":
            nc.sync.dma_start(out=outa[:, 0, :, 0, 0:8], in_=red8[:, :])
            nc.sync.dma_start(out=outa[:, 0, :, 1, 0:8], in_=gv8[:, :])
            nc.sync.dma_start(out=outa[:, 0, :, 2, 0:8], in_=pmax[:, 0:8])
            nc.sync.dma_start(out=outa[:, 0, :, 3, 0:8], in_=pmin[:, 0:8])

        # ---- phase 2: mask, dilate, boundary per (channel, 8-row quarter) ----
        def emit_thresh(idx):
            c, q = idx // 4, idx % 4
            binq = binp.tile([128, 10, W], bf16, tag="binq")
            if idx in t_dve:
                nc.vector.tensor_scalar(
                    out=binq[:, :, :],
                    in0=x_all[:, c, 8 * q : 8 * q + 10, :],
                    scalar1=mnv[:, c : c + 1],
                    scalar2=h4[:, c : c + 1],
                    op0=Alu.subtract,
                    op1=Alu.is_gt,
                )
            else:
                nc.scalar.activation(
                    out=binq[:, :, :],
                    in_=x_all[:, c, 8 * q : 8 * q + 10, :],
                    func=Act.Sigmoid,
                    bias=bias4[:, c : c + 1],
                    scale=1.0e8,
                )
            return binq

        def emit_rest(idx, binq):
            c, q = idx // 4, idx % 4
            mhq = mh[idx % 3]
            mhd = mhq[:, :, 1:257]
            nc.vector.tensor_tensor(
                out=mhd, in0=binq[:, 0:8, :], in1=binq[:, 2:10, :], op=Alu.max
            )
            nc.vector.tensor_tensor(
                out=mhd, in0=mhd, in1=binq[:, 1:9, :], op=Alu.max
            )
            ps = psump.tile([128, 2048], f32, tag="ps")
            if idx in w_pe:
                # W-dilation folded into PE: 3 dw-shifted band matmuls
                for s in range(4):
                    for j, dw in enumerate((0, 1, 2)):
                        nc.tensor.matmul(
                            ps[:, 512 * s : 512 * s + 512],
                            At[:, :],
                            mhq[:, 2 * s : 2 * s + 2, dw : dw + 256],
                            start=(j == 0),
                            stop=False,
                        )
            else:
                mwq = mwp.tile([128, 8, W], bf16, tag="mw")
                nc.vector.tensor_tensor(
                    out=mwq[:, :, :],
                    in0=mhq[:, :, 0:256],
                    in1=mhq[:, :, 2:258],
                    op=Alu.max,
                )
                nc.vector.tensor_tensor(
                    out=mwq[:, :, :], in0=mwq[:, :, :], in1=mhd, op=Alu.max
                )
                for s in range(4):
                    nc.tensor.matmul(
                        ps[:, 512 * s : 512 * s + 512],
                        At[:, :],
                        mwq[:, 2 * s : 2 * s + 2, :],
                        start=True,
                        stop=False,
                    )
            for s in range(4):
                nc.tensor.matmul(
                    ps[:, 512 * s : 512 * s + 512],
                    Nt[:, :],
                    binq[:, 2 * s + 1 : 2 * s + 3, :],
                    start=False,
                    stop=True,
                )
            sg = sgp.tile([128, 2048], f32, tag="sg")
            nc.scalar.activation(
                out=sg[:, :],
                in_=ps[:, :],
                func=Act.Sigmoid,
                bias=selb[:, :],
                scale=200.0,
            )
            nc.gpsimd.dma_start(
                out=outa[:, c, :, 8 * q : 8 * q + 8, :],
                in_=sg.rearrange("p (r w) -> p r w", w=W),
            )

        nquart = 16 if variant != "dbg" else 0
        prev = None
        for idx in range(nquart):
            binq = emit_thresh(idx)
            if prev is not None:
                emit_rest(*prev)
            prev = (idx, binq)
        if prev is not None:
            emit_rest(*prev)

    nc.compile()
    return nc


def _get_nc_single():
    if "nc1" not in _CACHE:
        _CACHE["nc1"] = _build()
    return _CACHE["nc1"]


def _make_in_maps(x: np.ndarray):
    in_maps = []
    for k in range(NCORES):
        xs = np.empty((B, C, D, HA, W), np.float32)
        lo = k * HS
        xs[:, :, :, 1 : HS + 1, :] = x[:, :, :, lo : lo + HS, :]
        if k > 0:
            xs[:, :, :, 0, :] = x[:, :, :, lo - 1, :]
        else:
            xs[:, :, :, 0, :] = HPAD
        if k < NCORES - 1:
            xs[:, :, :, HS + 1, :] = x[:, :, :, lo + HS, :]
        else:
            xs[:, :, :, HS + 1, :] = HPAD
        in_maps.append({"xs": xs})
    return in_maps


def kernel(x: np.ndarray) -> np.ndarray:
    from concourse.bass_utils import run_bass_kernel_spmd

    x = np.ascontiguousarray(np.asarray(x), dtype=np.float32)
    assert x.shape == (B, C, D, H, W)

    in_maps = _make_in_maps(x)
    res = run_bass_kernel_spmd(
        _get_nc_single(), in_maps, core_ids=list(range(NCORES))
    )
    pieces = [res.results[k]["out"] for k in range(NCORES)]
    return np.concatenate(pieces, axis=3)


if __name__ == "__main__":
    x = np.random.randn(B, C, D, H, W).astype(np.float32)
    y = kernel(x)
    print(y.shape, y.dtype, y.sum())
